# revision 19
# baseline (speedup 1.0000x reference)
"""Trainium2 Bass kernel for HOI detection head (nms_detection).

Reference computation per image b (B=8, N=100 boxes, 96x96x768 features):
  1. box-mean-pool features over each box region
  2. top-10 human boxes (label==1) and top-10 object boxes by score
  3. pairwise [human, object] concat -> MLP 1536->512->256->117 (relu x2)
  4. mask invalid pairs -> (rel [B,10,10,117], valid [B,10,10])

Strategy (data-parallel, 1 image per NeuronCore):
  - Host: top-k selection on (labels, scores) [tiny]; build an
    area-normalized box-indicator matrix A [20, 9216] so pooling is a
    matmul; relayout MLP weights for the PE's lhsT convention.
  - Device: pooledT[768,20] = sum_k F_k[128p,768].T-slices @ A_k[128p,20]
    (contraction over 9216 pixels, 72 tiles of 128), then the whole MLP in
    a transposed layout (activations [feat, pair]) so no on-device
    transposes are needed. Output relT [117, 100] per core.
  - Host: transpose/reshape, apply validity mask.
"""

import numpy as np

import concourse.bass as bass
import concourse.tile as tile
from concourse import bacc, mybir
from concourse.bass_utils import run_bass_kernel_spmd

F32 = mybir.dt.float32
RELU = mybir.ActivationFunctionType.Relu

B, N, HW, D = 8, 100, 96, 768
K = 10
NPIX = HW * HW          # 9216
NKT = NPIX // 128       # 72 pixel tiles
KT_PER_DMA = 4          # pixel tiles per feature DMA (1.5 MB each)
PERSON_ID = 1
NEG = np.float32(-1e30)

_CACHE = {}


def _build_program():
    # Bacc (not plain Bass): its compile() runs generate_event_semaphores,
    # which splits multi-semaphore waits that walrus codegen rejects.
    nc = bacc.Bacc("TRN2", target_bir_lowering=False, debug=False, num_devices=B)

    f_d = nc.declare_dram_parameter("f", [NPIX, D], F32, isOutput=False)
    at_d = nc.declare_dram_parameter("at", [128, NKT * 20], F32, isOutput=False)
    w1_d = nc.declare_dram_parameter("w1", [128, 12 * 512], F32, isOutput=False)
    w2_d = nc.declare_dram_parameter("w2", [128, 8 * 128], F32, isOutput=False)
    w3_d = nc.declare_dram_parameter("w3", [128, 2 * 117], F32, isOutput=False)
    b1_d = nc.declare_dram_parameter("b1", [1, 512], F32, isOutput=False)
    b2_d = nc.declare_dram_parameter("b2", [1, 256], F32, isOutput=False)
    b3_d = nc.declare_dram_parameter("b3", [1, 117], F32, isOutput=False)
    sth_d = nc.declare_dram_parameter("sth", [10, 100], F32, isOutput=False)
    sto_d = nc.declare_dram_parameter("sto", [10, 100], F32, isOutput=False)
    ones_d = nc.declare_dram_parameter("ones", [1, 100], F32, isOutput=False)
    o_d = nc.declare_dram_parameter("o", [117, 100], F32, isOutput=True)

    with tile.TileContext(nc) as tc:
        with (
            tc.tile_pool(name="fpool", bufs=8) as fpool,
            tc.tile_pool(name="const", bufs=1) as const,
            tc.tile_pool(name="sb", bufs=1) as sb,
            tc.tile_pool(name="psum", bufs=1, space="PSUM") as pp,
        ):
            at_sb = const.tile([128, NKT * 20], F32)
            nc.gpsimd.dma_start(at_sb[:], at_d[:])
            w1sb = const.tile([128, 12 * 512], F32)
            nc.gpsimd.dma_start(w1sb[:], w1_d[:])
            w2sb = const.tile([128, 8 * 128], F32)
            nc.gpsimd.dma_start(w2sb[:], w2_d[:])
            w3sb = const.tile([128, 2 * 117], F32)
            nc.gpsimd.dma_start(w3sb[:], w3_d[:])
            b1sb = const.tile([1, 512], F32)
            nc.gpsimd.dma_start(b1sb[:], b1_d[:])
            b2sb = const.tile([1, 256], F32)
            nc.gpsimd.dma_start(b2sb[:], b2_d[:])
            b3sb = const.tile([1, 117], F32)
            nc.gpsimd.dma_start(b3sb[:], b3_d[:])
            sth_sb = const.tile([10, 100], F32)
            nc.gpsimd.dma_start(sth_sb[:], sth_d[:])
            sto_sb = const.tile([10, 100], F32)
            nc.gpsimd.dma_start(sto_sb[:], sto_d[:])
            ones_sb = const.tile([1, 100], F32)
            nc.gpsimd.dma_start(ones_sb[:], ones_d[:])

            # ---- Stage 1: pooledT[768, 20] as 6 chunks [128, 20] in one PSUM bank
            # Warmup matmul reading only at_sb: a PE instruction supports a
            # single semaphore wait, so absorb the at_sb DMA wait here and the
            # first real matmul then only waits on its feature-tile DMA.
            junk = pp.tile([1, 1], F32, name="junk", tag="mlp", bufs=2)
            nc.tensor.matmul(junk[:], lhsT=at_sb[0:1, 0:1], rhs=at_sb[0:1, 0:1],
                             start=True, stop=True)
            # One PSUM bank per accumulation group (a start=True zeroes the
            # whole bank region, so groups can't share a bank).
            pdc = [pp.tile([128, 20], F32, name=f"pd{c}", tag="pd", bufs=6)
                   for c in range(6)]
            nkd = NKT // KT_PER_DMA
            for kd in range(nkd):
                ft = fpool.tile([128, KT_PER_DMA * D], F32)
                # sbuf[p, t*768+j] = f[(kd*KT+t)*128 + p, j]
                nc.gpsimd.dma_start(
                    ft[:].rearrange("p (t j) -> p t j", t=KT_PER_DMA),
                    f_d[kd * KT_PER_DMA * 128:(kd + 1) * KT_PER_DMA * 128, :]
                    .rearrange("(t p) j -> p t j", p=128),
                )
                for t in range(KT_PER_DMA):
                    k = kd * KT_PER_DMA + t
                    for c in range(6):
                        nc.tensor.matmul(
                            pdc[c][:],
                            lhsT=ft[:, t * D + c * 128: t * D + (c + 1) * 128],
                            rhs=at_sb[:, k * 20:(k + 1) * 20],
                            start=(k == 0),
                            stop=(k == NKT - 1),
                        )

            pooledT = sb.tile([128, 6 * 20], F32)
            for c in range(6):
                nc.scalar.copy(pooledT[:, c * 20:(c + 1) * 20], pdc[c][:])

            # ---- Stage 2: G' [21, 512] = [hfeat@W1_top; ofeat@W1_bot; b1]
            pgh = pp.tile([10, 512], F32, name="pgh", tag="mlp", bufs=2)
            pgo = pp.tile([10, 512], F32, name="pgo", tag="mlp", bufs=2)
            for c in range(6):
                nc.tensor.matmul(
                    pgh[:],
                    lhsT=pooledT[:, c * 20: c * 20 + 10],
                    rhs=w1sb[:, c * 512:(c + 1) * 512],
                    start=(c == 0), stop=(c == 5),
                )
                nc.tensor.matmul(
                    pgo[:],
                    lhsT=pooledT[:, c * 20 + 10: c * 20 + 20],
                    rhs=w1sb[:, (6 + c) * 512:(7 + c) * 512],
                    start=(c == 0), stop=(c == 5),
                )
            g_h = sb.tile([10, 512], F32)
            nc.scalar.copy(g_h[:], pgh[:])
            g_o = sb.tile([10, 512], F32)
            nc.scalar.copy(g_o[:], pgo[:])

            # ---- Stage 3: x1T [512, 100] as [128, 4*100]
            # x1_preT = G_h.T @ ShT + G_o.T @ SoT + b1.T @ ones
            x1 = sb.tile([128, 4 * 100], F32)
            for m in range(4):
                px1 = pp.tile([128, 100], F32, name=f"px1_{m}", tag="mlp", bufs=2)
                sl = slice(m * 128, (m + 1) * 128)
                nc.tensor.matmul(px1[:], lhsT=g_h[:, sl], rhs=sth_sb[:],
                                 start=True, stop=False)
                nc.tensor.matmul(px1[:], lhsT=g_o[:, sl], rhs=sto_sb[:],
                                 start=False, stop=False)
                nc.tensor.matmul(px1[:], lhsT=b1sb[:, sl], rhs=ones_sb[:],
                                 start=False, stop=True)
                nc.scalar.activation(x1[:, m * 100:(m + 1) * 100], px1[:], RELU)

            # ---- Stage 4: x2T [256, 100] = relu(W2.T @ x1 + b2) as [128, 2*100]
            x2 = sb.tile([128, 2 * 100], F32)
            for n in range(2):
                px2 = pp.tile([128, 100], F32, name=f"px2_{n}", tag="mlp", bufs=2)
                for m in range(4):
                    nc.tensor.matmul(
                        px2[:],
                        lhsT=w2sb[:, (m * 2 + n) * 128:(m * 2 + n + 1) * 128],
                        rhs=x1[:, m * 100:(m + 1) * 100],
                        start=(m == 0), stop=False,
                    )
                nc.tensor.matmul(
                    px2[:],
                    lhsT=b2sb[:, n * 128:(n + 1) * 128],
                    rhs=ones_sb[:],
                    start=False, stop=True,
                )
                nc.scalar.activation(x2[:, n * 100:(n + 1) * 100], px2[:], RELU)

            # ---- Stage 5: relT [117, 100] = W3.T @ x2 + b3
            pr = pp.tile([117, 100], F32, name="pr", tag="mlp", bufs=2)
            for m in range(2):
                nc.tensor.matmul(
                    pr[:],
                    lhsT=w3sb[:, m * 117:(m + 1) * 117],
                    rhs=x2[:, m * 100:(m + 1) * 100],
                    start=(m == 0), stop=False,
                )
            nc.tensor.matmul(
                pr[:], lhsT=b3sb[:], rhs=ones_sb[:], start=False, stop=True,
            )
            osb = sb.tile([117, 100], F32)
            nc.scalar.copy(osb[:], pr[:])
            nc.gpsimd.dma_start(o_d[:], osb[:])

    nc.compile()
    return nc


def _host_prep(features, boxes, labels, scores, W1, b1, W2, b2, W3, b3):
    """Top-k selection + pooling-matrix construction + weight relayout."""
    h_sc = np.where(labels == PERSON_ID, scores, NEG).astype(np.float32)
    o_sc = np.where(labels != PERSON_ID, scores, NEG).astype(np.float32)
    hidx = np.argsort(-h_sc, axis=1, kind="stable")[:, :K]
    oidx = np.argsort(-o_sc, axis=1, kind="stable")[:, :K]
    h_ok = np.take_along_axis(h_sc, hidx, 1) > NEG / 2
    o_ok = np.take_along_axis(o_sc, oidx, 1) > NEG / 2
    valid = h_ok[:, :, None] & o_ok[:, None, :]          # [B, K, K]

    # A [B, 20, 9216]: area-normalized box indicators over flattened (y, x)
    at_maps = []
    for b in range(B):
        sel = np.concatenate([hidx[b], oidx[b]])          # [20]
        A = np.zeros((2 * K, HW, HW), np.float32)
        for j, n in enumerate(sel):
            x1_, y1_, x2_, y2_ = boxes[b, n]
            area = float((y2_ - y1_) * (x2_ - x1_))
            A[j, y1_:y2_, x1_:x2_] = 1.0 / area
        # at[p, k*20+j] = A[j, k*128+p]
        at = np.ascontiguousarray(
            A.reshape(2 * K, NKT, 128).transpose(2, 1, 0).reshape(128, NKT * 20)
        )
        at_maps.append(at)

    w1sb = np.ascontiguousarray(
        W1.reshape(12, 128, 512).transpose(1, 0, 2).reshape(128, 12 * 512)
    )
    w2sb = np.ascontiguousarray(
        W2.reshape(4, 128, 2, 128).transpose(1, 0, 2, 3).reshape(128, 8 * 128)
    )
    w3sb = np.ascontiguousarray(
        W3.reshape(2, 128, 117).transpose(1, 0, 2).reshape(128, 2 * 117)
    )

    # ShT/SoT [10, 100]: ShT[i, p] = (p//10 == i), SoT[j, p] = (p%10 == j)
    p = np.arange(K * K)
    sth = np.zeros((K, K * K), np.float32)
    sth[p // K, p] = 1.0
    sto = np.zeros((K, K * K), np.float32)
    sto[p % K, p] = 1.0

    shared = dict(
        w1=w1sb, w2=w2sb, w3=w3sb,
        b1=np.ascontiguousarray(b1.reshape(1, 512)),
        b2=np.ascontiguousarray(b2.reshape(1, 256)),
        b3=np.ascontiguousarray(b3.reshape(1, 117)),
        sth=sth, sto=sto, ones=np.ones((1, 100), np.float32),
    )
    in_maps = []
    for b in range(B):
        m = dict(shared)
        m["f"] = np.ascontiguousarray(features[b].reshape(NPIX, D))
        m["at"] = at_maps[b]
        in_maps.append(m)
    return in_maps, valid


def _get_program():
    if "nc" not in _CACHE:
        _CACHE["nc"] = _build_program()
    return _CACHE["nc"]


def run(features, boxes, labels, scores, W1, b1, W2, b2, W3, b3, **spmd_kwargs):
    """Full pipeline; returns ((rel, valid), BassKernelResults)."""
    in_maps, valid = _host_prep(
        features, boxes, labels, scores, W1, b1, W2, b2, W3, b3
    )
    nc = _get_program()
    res = run_bass_kernel_spmd(nc, in_maps, list(range(B)), **spmd_kwargs)
    rel = np.zeros((B, K, K, 117), np.float32)
    for b in range(B):
        o = res.results[b]["o"]                           # [117, 100]
        rel[b] = o.T.reshape(K, K, 117) * valid[b][:, :, None]
    return (rel, valid), res


def kernel(features, boxes, labels, scores, W1, b1, W2, b2, W3, b3):
    out, _ = run(features, boxes, labels, scores, W1, b1, W2, b2, W3, b3)
    return out


# revision 22
# speedup vs baseline: 1.5792x; 1.5792x over previous
"""Trainium2 Bass kernel for HOI detection head (nms_detection).

Reference computation per image b (B=8, N=100 boxes, 96x96x768 features):
  1. box-mean-pool features over each box region
  2. top-10 human boxes (label==1) and top-10 object boxes by score
  3. pairwise [human, object] concat -> MLP 1536->512->256->117 (relu x2)
  4. mask invalid pairs -> (rel [B,10,10,117], valid [B,10,10])

Strategy (data-parallel, 1 image per NeuronCore):
  - Host: top-k selection on (labels, scores) [tiny]; build an
    area-normalized box-indicator matrix A [20, 9216] so pooling is a
    matmul; relayout MLP weights for the PE's lhsT convention.
  - Device: pooledT[768,20] = sum_k F_k[128p,768].T-slices @ A_k[128p,20]
    (contraction over 9216 pixels, 72 tiles of 128), then the whole MLP in
    a transposed layout (activations [feat, pair]) so no on-device
    transposes are needed. Output relT [117, 100] per core.
  - Host: transpose/reshape, apply validity mask.
"""

import numpy as np

import concourse.bass as bass
import concourse.tile as tile
from concourse import bacc, mybir
from concourse.bass_utils import run_bass_kernel_spmd

F32 = mybir.dt.float32
F32R = mybir.dt.float32r   # same bits as f32; 4x faster PE path for pooling
RELU = mybir.ActivationFunctionType.Relu

B, N, HW, D = 8, 100, 96, 768
K = 10
NPIX = HW * HW          # 9216
NKT = NPIX // 128       # 72 pixel tiles
KT_PER_DMA = 4          # pixel tiles per feature DMA (1.5 MB each)
PERSON_ID = 1
NEG = np.float32(-1e30)

_CACHE = {}


def _build_program():
    # Bacc (not plain Bass): its compile() runs generate_event_semaphores,
    # which splits multi-semaphore waits that walrus codegen rejects.
    nc = bacc.Bacc("TRN2", target_bir_lowering=False, debug=False, num_devices=B)

    f_d = nc.declare_dram_parameter("f", [NPIX, D], F32R, isOutput=False)
    at_d = nc.declare_dram_parameter("at", [128, NKT * 20], F32R, isOutput=False)
    w1_d = nc.declare_dram_parameter("w1", [128, 12 * 512], F32, isOutput=False)
    w2_d = nc.declare_dram_parameter("w2", [128, 8 * 128], F32, isOutput=False)
    w3_d = nc.declare_dram_parameter("w3", [128, 2 * 117], F32, isOutput=False)
    b1_d = nc.declare_dram_parameter("b1", [1, 512], F32, isOutput=False)
    b2_d = nc.declare_dram_parameter("b2", [1, 256], F32, isOutput=False)
    b3_d = nc.declare_dram_parameter("b3", [1, 117], F32, isOutput=False)
    sth_d = nc.declare_dram_parameter("sth", [10, 100], F32, isOutput=False)
    sto_d = nc.declare_dram_parameter("sto", [10, 100], F32, isOutput=False)
    ones_d = nc.declare_dram_parameter("ones", [1, 100], F32, isOutput=False)
    o_d = nc.declare_dram_parameter("o", [117, 100], F32, isOutput=True)

    with tile.TileContext(nc) as tc:
        with (
            tc.tile_pool(name="fpool", bufs=8) as fpool,
            tc.tile_pool(name="const", bufs=1) as const,
            tc.tile_pool(name="sb", bufs=1) as sb,
            tc.tile_pool(name="psum", bufs=1, space="PSUM") as pp,
        ):
            at_sb = const.tile([128, NKT * 20], F32R)
            nc.gpsimd.dma_start(at_sb[:], at_d[:])
            w1sb = const.tile([128, 12 * 512], F32)
            nc.gpsimd.dma_start(w1sb[:], w1_d[:])
            w2sb = const.tile([128, 8 * 128], F32)
            nc.gpsimd.dma_start(w2sb[:], w2_d[:])
            w3sb = const.tile([128, 2 * 117], F32)
            nc.gpsimd.dma_start(w3sb[:], w3_d[:])
            b1sb = const.tile([1, 512], F32)
            nc.gpsimd.dma_start(b1sb[:], b1_d[:])
            b2sb = const.tile([1, 256], F32)
            nc.gpsimd.dma_start(b2sb[:], b2_d[:])
            b3sb = const.tile([1, 117], F32)
            nc.gpsimd.dma_start(b3sb[:], b3_d[:])
            sth_sb = const.tile([10, 100], F32)
            nc.gpsimd.dma_start(sth_sb[:], sth_d[:])
            sto_sb = const.tile([10, 100], F32)
            nc.gpsimd.dma_start(sto_sb[:], sto_d[:])
            ones_sb = const.tile([1, 100], F32)
            nc.gpsimd.dma_start(ones_sb[:], ones_d[:])

            # ---- Stage 1: pooledT[768, 20] as 6 chunks [128, 20] in one PSUM bank
            # Warmup matmul reading only at_sb: a PE instruction supports a
            # single semaphore wait, so absorb the at_sb DMA wait here and the
            # first real matmul then only waits on its feature-tile DMA.
            junk = pp.tile([2, 16], F32, name="junk", tag="mlp", bufs=2)
            nc.tensor.matmul(junk[:], lhsT=at_sb[0:2, 0:2], rhs=at_sb[0:2, 0:16],
                             start=True, stop=True)
            # One PSUM bank per accumulation group (a start=True zeroes the
            # whole bank region, so groups can't share a bank).
            pdc = [pp.tile([128, 20], F32, name=f"pd{c}", tag="pd", bufs=6)
                   for c in range(6)]
            nkd = NKT // KT_PER_DMA
            for kd in range(nkd):
                ft = fpool.tile([128, KT_PER_DMA * D], F32R)
                # sbuf[p, t*768+j] = f[(kd*KT+t)*128 + p, j]
                nc.gpsimd.dma_start(
                    ft[:].rearrange("p (t j) -> p t j", t=KT_PER_DMA),
                    f_d[kd * KT_PER_DMA * 128:(kd + 1) * KT_PER_DMA * 128, :]
                    .rearrange("(t p) j -> p t j", p=128),
                )
                for t in range(KT_PER_DMA):
                    k = kd * KT_PER_DMA + t
                    for c in range(6):
                        nc.tensor.matmul(
                            pdc[c][:],
                            lhsT=ft[:, t * D + c * 128: t * D + (c + 1) * 128],
                            rhs=at_sb[:, k * 20:(k + 1) * 20],
                            start=(k == 0),
                            stop=(k == NKT - 1),
                        )

            pooledT = sb.tile([128, 6 * 20], F32)
            for c in range(6):
                nc.scalar.copy(pooledT[:, c * 20:(c + 1) * 20], pdc[c][:])

            # ---- Stage 2: G' [21, 512] = [hfeat@W1_top; ofeat@W1_bot; b1]
            pgh = pp.tile([10, 512], F32, name="pgh", tag="mlp", bufs=2)
            pgo = pp.tile([10, 512], F32, name="pgo", tag="mlp", bufs=2)
            for c in range(6):
                nc.tensor.matmul(
                    pgh[:],
                    lhsT=pooledT[:, c * 20: c * 20 + 10],
                    rhs=w1sb[:, c * 512:(c + 1) * 512],
                    start=(c == 0), stop=(c == 5),
                )
                nc.tensor.matmul(
                    pgo[:],
                    lhsT=pooledT[:, c * 20 + 10: c * 20 + 20],
                    rhs=w1sb[:, (6 + c) * 512:(7 + c) * 512],
                    start=(c == 0), stop=(c == 5),
                )
            g_h = sb.tile([10, 512], F32)
            nc.scalar.copy(g_h[:], pgh[:])
            g_o = sb.tile([10, 512], F32)
            nc.scalar.copy(g_o[:], pgo[:])

            # ---- Stage 3: x1T [512, 100] as [128, 4*100]
            # x1_preT = G_h.T @ ShT + G_o.T @ SoT + b1.T @ ones
            x1 = sb.tile([128, 4 * 100], F32)
            for m in range(4):
                px1 = pp.tile([128, 100], F32, name=f"px1_{m}", tag="mlp", bufs=2)
                sl = slice(m * 128, (m + 1) * 128)
                nc.tensor.matmul(px1[:], lhsT=g_h[:, sl], rhs=sth_sb[:],
                                 start=True, stop=False)
                nc.tensor.matmul(px1[:], lhsT=g_o[:, sl], rhs=sto_sb[:],
                                 start=False, stop=False)
                nc.tensor.matmul(px1[:], lhsT=b1sb[:, sl], rhs=ones_sb[:],
                                 start=False, stop=True)
                nc.scalar.activation(x1[:, m * 100:(m + 1) * 100], px1[:], RELU)

            # ---- Stage 4: x2T [256, 100] = relu(W2.T @ x1 + b2) as [128, 2*100]
            x2 = sb.tile([128, 2 * 100], F32)
            for n in range(2):
                px2 = pp.tile([128, 100], F32, name=f"px2_{n}", tag="mlp", bufs=2)
                for m in range(4):
                    nc.tensor.matmul(
                        px2[:],
                        lhsT=w2sb[:, (m * 2 + n) * 128:(m * 2 + n + 1) * 128],
                        rhs=x1[:, m * 100:(m + 1) * 100],
                        start=(m == 0), stop=False,
                    )
                nc.tensor.matmul(
                    px2[:],
                    lhsT=b2sb[:, n * 128:(n + 1) * 128],
                    rhs=ones_sb[:],
                    start=False, stop=True,
                )
                nc.scalar.activation(x2[:, n * 100:(n + 1) * 100], px2[:], RELU)

            # ---- Stage 5: relT [117, 100] = W3.T @ x2 + b3
            pr = pp.tile([117, 100], F32, name="pr", tag="mlp", bufs=2)
            for m in range(2):
                nc.tensor.matmul(
                    pr[:],
                    lhsT=w3sb[:, m * 117:(m + 1) * 117],
                    rhs=x2[:, m * 100:(m + 1) * 100],
                    start=(m == 0), stop=False,
                )
            nc.tensor.matmul(
                pr[:], lhsT=b3sb[:], rhs=ones_sb[:], start=False, stop=True,
            )
            osb = sb.tile([117, 100], F32)
            nc.scalar.copy(osb[:], pr[:])
            nc.gpsimd.dma_start(o_d[:], osb[:])

    nc.compile()
    return nc


def _host_prep(features, boxes, labels, scores, W1, b1, W2, b2, W3, b3):
    """Top-k selection + pooling-matrix construction + weight relayout."""
    h_sc = np.where(labels == PERSON_ID, scores, NEG).astype(np.float32)
    o_sc = np.where(labels != PERSON_ID, scores, NEG).astype(np.float32)
    hidx = np.argsort(-h_sc, axis=1, kind="stable")[:, :K]
    oidx = np.argsort(-o_sc, axis=1, kind="stable")[:, :K]
    h_ok = np.take_along_axis(h_sc, hidx, 1) > NEG / 2
    o_ok = np.take_along_axis(o_sc, oidx, 1) > NEG / 2
    valid = h_ok[:, :, None] & o_ok[:, None, :]          # [B, K, K]

    # A [B, 20, 9216]: area-normalized box indicators over flattened (y, x)
    at_maps = []
    for b in range(B):
        sel = np.concatenate([hidx[b], oidx[b]])          # [20]
        A = np.zeros((2 * K, HW, HW), np.float32)
        for j, n in enumerate(sel):
            x1_, y1_, x2_, y2_ = boxes[b, n]
            area = float((y2_ - y1_) * (x2_ - x1_))
            A[j, y1_:y2_, x1_:x2_] = 1.0 / area
        # at[p, k*20+j] = A[j, k*128+p]
        at = np.ascontiguousarray(
            A.reshape(2 * K, NKT, 128).transpose(2, 1, 0).reshape(128, NKT * 20)
        )
        at_maps.append(at)

    w1sb = np.ascontiguousarray(
        W1.reshape(12, 128, 512).transpose(1, 0, 2).reshape(128, 12 * 512)
    )
    w2sb = np.ascontiguousarray(
        W2.reshape(4, 128, 2, 128).transpose(1, 0, 2, 3).reshape(128, 8 * 128)
    )
    w3sb = np.ascontiguousarray(
        W3.reshape(2, 128, 117).transpose(1, 0, 2).reshape(128, 2 * 117)
    )

    # ShT/SoT [10, 100]: ShT[i, p] = (p//10 == i), SoT[j, p] = (p%10 == j)
    p = np.arange(K * K)
    sth = np.zeros((K, K * K), np.float32)
    sth[p // K, p] = 1.0
    sto = np.zeros((K, K * K), np.float32)
    sto[p % K, p] = 1.0

    shared = dict(
        w1=w1sb, w2=w2sb, w3=w3sb,
        b1=np.ascontiguousarray(b1.reshape(1, 512)),
        b2=np.ascontiguousarray(b2.reshape(1, 256)),
        b3=np.ascontiguousarray(b3.reshape(1, 117)),
        sth=sth, sto=sto, ones=np.ones((1, 100), np.float32),
    )
    in_maps = []
    for b in range(B):
        m = dict(shared)
        m["f"] = np.ascontiguousarray(features[b].reshape(NPIX, D))
        m["at"] = at_maps[b]
        in_maps.append(m)
    return in_maps, valid


def _get_program():
    if "nc" not in _CACHE:
        _CACHE["nc"] = _build_program()
    return _CACHE["nc"]


def run(features, boxes, labels, scores, W1, b1, W2, b2, W3, b3, **spmd_kwargs):
    """Full pipeline; returns ((rel, valid), BassKernelResults)."""
    in_maps, valid = _host_prep(
        features, boxes, labels, scores, W1, b1, W2, b2, W3, b3
    )
    nc = _get_program()
    res = run_bass_kernel_spmd(nc, in_maps, list(range(B)), **spmd_kwargs)
    rel = np.zeros((B, K, K, 117), np.float32)
    for b in range(B):
        o = res.results[b]["o"]                           # [117, 100]
        rel[b] = o.T.reshape(K, K, 117) * valid[b][:, :, None]
    return (rel, valid), res


def kernel(features, boxes, labels, scores, W1, b1, W2, b2, W3, b3):
    out, _ = run(features, boxes, labels, scores, W1, b1, W2, b2, W3, b3)
    return out
